# revision 2
# baseline (speedup 1.0000x reference)
"""DRMGCN (dual-branch 5-layer GCN + channel attention + outer product)
on 8 TRN2 NeuronCores.

Strategy
--------
- Graph aggregation cast as dense matmul against the normalized
  adjacency (random graph => no usable sparsity): agg = A_hat @ z, with
  A_hat built on host, padded to 10240 nodes, stored transposed
  (src-major). Nodes sharded 8-way (1280/core); each layer computes
  z = h @ W locally, AllGathers z, then contracts against the core's
  A_hat.T column slice on the tensor engine. h kept transposed [f, n]
  on-chip so every matmul in the chain is transpose-free.
- Precision plan (measured): quantization error injected at layer 0 is
  amplified ~3x through the remaining layers, while error injected at
  layers 1-4 is nearly free. So layer 0 runs bf16 x bf16, and layers
  1-4 run fully fp8e4 (A_hat AND z) with DoubleRow perf mode: 2
  contraction rows/cycle (~1.44x tensor) and half the A_hat DMA.
  Scale folding keeps every fp8 operand in e4m3's normal range:
  A_hat x32 on host, z boosted x4 via the per-layer W, and the 1/128
  net scale divided out for free in the relu activation's scale field.
- Software-pipelined layer schedule: each branch's z+AllGather is
  emitted BEFORE the other branch's agg matmul stream, so the ~12us
  collective latency always hides under the ~55-85us agg.
- Split per-branch attention AllReduce; branch 1 (disease) leads so its
  conv + conv-output AllGather hide under branch-0 tail work.
- Final [1280,128]@[128,10240] row-shard product: all 8 gathered kt
  tiles SBUF-resident, output accumulated into [128, 10240] row tiles
  (PSUM evictions alternate vector/scalar engines), one contiguous
  2.6MB DMA per row tile.
- All-zero padding chunk (rows 10112+) skipped in non-DR k-loops.
"""

import numpy as np
import ml_dtypes

import concourse.mybir as mybir
import concourse.tile as tile
from concourse import bacc
from concourse.bass_utils import run_bass_kernel_spmd

NC = 8          # cores
N_NODE = 10000  # real nodes per branch
NPAD = 10240    # padded (multiple of 8*128)
S = NPAD // NC  # 1280 nodes per core
P = 128
SM = S // P     # 10 m-tiles per shard
F = 256         # feature dim
FC = F // P     # 2 feature chunks
L = 5           # gcn layers
OC = 128        # conv out channels
KC = NPAD // P  # 80 contraction chunks
KC2 = KC // 2   # 40 double-row chunks
SM2 = SM // 2
NT = [(0, 512), (512, 512), (1024, 256)]  # n-tiles within a 1280 shard
ASCALE = 32.0   # A_hat scale folded into W (e3m4 normal range)
ZSCALE = 4.0    # extra z boost for DR layers (e4m3 normal range)

F32 = mybir.dt.float32
BF16 = mybir.dt.bfloat16
FP16 = mybir.dt.float16
FP8E4 = mybir.dt.float8e4
FP8E3 = mybir.dt.float8e3
BF = ml_dtypes.bfloat16
E4 = ml_dtypes.float8_e4m3
E3 = ml_dtypes.float8_e3m4
AF = mybir.ActivationFunctionType
RG = [list(range(NC))]

DR = False      # legacy global DoubleRow mode (ablation only)
# (br, i) instances whose A_hat matmul uses fp8e3 with bf16 z (no DoubleRow)
FP8_LI = ()
# (br, i) instances run fully fp8e4 (A and z) via DoubleRow. Layer-0
# instances must stay bf16: early-layer quantization error is amplified
# ~3x through the stack (measured); late-layer error is ~free.
DR_LI = tuple((br, i) for br in range(2) for i in range(1, L))

_CACHE = {}


def _build(repeat=1, timing=False, only_layers=False, dr=None,
           skip_ag=False, skip_z=False, skip_at=False, at_bufs=8, zk_bufs=6):
    dr = DR if dr is None else dr
    fp8_li = frozenset(FP8_LI)
    dr_li = frozenset() if dr else frozenset(DR_LI)
    all_li = {(br, i) for br in range(2) for i in range(L)}
    need16 = len(all_li - fp8_li - dr_li) > 0
    need8 = len(fp8_li) > 0
    needdr = len(dr_li) > 0
    nc = bacc.Bacc("TRN2", target_bir_lowering=False, debug=False, num_devices=NC)

    at_d, x0t_d, w_d, bt_d, cwt_d, cb_d = [], [], [], [], [], []
    fc1wt_d, fc1b_d, fc2wt_d, fc2b_d = [], [], [], []
    for br in range(2):
        if dr:
            at_d.append(nc.dram_tensor(f"at{br}", [KC2, P, 2, S], FP8E4,
                                       kind="ExternalInput"))
        else:
            trio = [None, None, None]
            if need16:
                trio[0] = nc.dram_tensor(f"at{br}", [KC, P, S], BF16,
                                         kind="ExternalInput")
            if need8:
                trio[1] = nc.dram_tensor(f"at8_{br}", [KC, P, S], FP8E3,
                                         kind="ExternalInput")
            if needdr:
                trio[2] = nc.dram_tensor(f"atdr{br}", [KC2, P, 2, S], FP8E4,
                                         kind="ExternalInput")
            at_d.append(trio)
        x0t_d.append(nc.dram_tensor(f"x0t{br}", [FC, P, S], BF16, kind="ExternalInput"))
        w_d.append(nc.dram_tensor(f"w{br}", [L, FC, P, F], BF16, kind="ExternalInput"))
        bt_d.append(nc.dram_tensor(f"bt{br}", [L, FC, P], F32, kind="ExternalInput"))
        cwt_d.append(nc.dram_tensor(f"cwt{br}", [L, FC, P, OC], BF16, kind="ExternalInput"))
        cb_d.append(nc.dram_tensor(f"cb{br}", [P, 1], F32, kind="ExternalInput"))
        fc1wt_d.append(nc.dram_tensor(f"fc1wt{br}", [L, 5 * L], F32, kind="ExternalInput"))
        fc1b_d.append(nc.dram_tensor(f"fc1b{br}", [5 * L, 1], F32, kind="ExternalInput"))
        fc2wt_d.append(nc.dram_tensor(f"fc2wt{br}", [5 * L, L], F32, kind="ExternalInput"))
        fc2b_d.append(nc.dram_tensor(f"fc2b{br}", [L, 1], F32, kind="ExternalInput"))
    if timing:
        done_d = nc.dram_tensor("done", [P, 2 * L], F32, kind="ExternalOutput")
        out_d = None
    else:
        out_d = nc.dram_tensor("out", [S, NPAD], FP16, kind="ExternalOutput")

    with tile.TileContext(nc) as tc:
        with (
            tc.tile_pool(name="const", bufs=1) as const,
            tc.tile_pool(name="sb", bufs=2) as sb,
            tc.tile_pool(name="zsb", bufs=2) as zsb,
            tc.tile_pool(name="zk", bufs=zk_bufs) as zkp,
            tc.tile_pool(name="atp", bufs=at_bufs) as atp,
            tc.tile_pool(name="ktp", bufs=8) as ktp,
            tc.tile_pool(name="fop", bufs=2) as fop,
            tc.tile_pool(name="psa", bufs=6, space="PSUM") as psa,
            tc.tile_pool(name="psz", bufs=2, space="PSUM") as psz,
            tc.tile_pool(name="dram", bufs=2, space="DRAM") as dram,
        ):
            mx_sb = const.tile([P, 2 * L], F32, name="mx_sb")
            nc.vector.memset(mx_sb[:], 0.0)
            ones_sb = const.tile([1, P], F32, name="ones_sb")
            nc.vector.memset(ones_sb[:], 1.0)

            w_sb, bt_sb, cwt_sb, cb_sb = [], [], [], []
            fc1wt_sb, fc1b_sb, fc2wt_sb, fc2b_sb = [], [], [], []
            for br in range(2):
                w_t = const.tile([P, L, FC, F], BF16, name=f"w_sb{br}")
                cw_t = const.tile([P, L, FC, OC], BF16, name=f"cwt_sb{br}")
                for l in range(L):
                    nc.sync.dma_start(w_t[:, l], w_d[br][l].rearrange("fc p f -> p fc f"))
                    nc.sync.dma_start(cw_t[:, l], cwt_d[br][l].rearrange("fc p o -> p fc o"))
                bt_t = const.tile([P, L, FC], F32, name=f"bt_sb{br}")
                nc.sync.dma_start(bt_t[:], bt_d[br].rearrange("l fc p -> p l fc"))
                cb_t = const.tile([P, 1], F32, name=f"cb_sb{br}")
                nc.sync.dma_start(cb_t[:], cb_d[br][:])
                f1w = const.tile([L, 5 * L], F32, name=f"fc1wt_sb{br}")
                nc.sync.dma_start(f1w[:], fc1wt_d[br][:])
                f1b = const.tile([5 * L, 1], F32, name=f"fc1b_sb{br}")
                nc.sync.dma_start(f1b[:], fc1b_d[br][:])
                f2w = const.tile([5 * L, L], F32, name=f"fc2wt_sb{br}")
                nc.sync.dma_start(f2w[:], fc2wt_d[br][:])
                f2b = const.tile([L, 1], F32, name=f"fc2b_sb{br}")
                nc.sync.dma_start(f2b[:], fc2b_d[br][:])
                w_sb.append(w_t); bt_sb.append(bt_t); cwt_sb.append(cw_t); cb_sb.append(cb_t)
                fc1wt_sb.append(f1w); fc1b_sb.append(f1b); fc2wt_sb.append(f2w); fc2b_sb.append(f2b)

            x0t_sb = []
            for br in range(2):
                x0t_t = const.tile([P, FC, S], BF16, name=f"x0t_sb{br}")
                nc.sync.dma_start(x0t_t[:], x0t_d[br].rearrange("fc p s -> p fc s"))
                x0t_sb.append(x0t_t)

            if timing:
                outbig = dram.tile([S, NPAD], FP16, name="outbig", bufs=1)
                out_tgt = outbig
            else:
                out_tgt = out_d

            def emit():
              ht = [[None] * L, [None] * L]
              zcur = [None, None]   # (z_sbuf_tile, zf_dram) per branch

              def z_step(br, i):
                  """z = h @ W[i]; write + AllGather."""
                  drz = dr or (br, i) in dr_li
                  hprev = x0t_sb[br] if i == 0 else ht[br][i - 1]
                  if skip_z:
                      zcur[br] = (None, None)
                      return
                  if drz:
                      z_sb = zsb.tile([P, SM2, 2, F], FP8E4, name="z_sb")
                  else:
                      z_sb = zsb.tile([P, SM, F], BF16, name="z_sb")
                  for m in range(SM):
                      zp = psz.tile([P, F], F32, name="zp", tag="psz")
                      for fc in range(FC):
                          nc.tensor.matmul(
                              zp[:],
                              hprev[:, fc, m * P:(m + 1) * P],
                              w_sb[br][:, i, fc, :],
                              start=(fc == 0),
                              stop=(fc == FC - 1),
                          )
                      if drz:
                          nc.vector.tensor_copy(z_sb[:, m // 2, m % 2, :], zp[:])
                      else:
                          nc.vector.tensor_copy(z_sb[:, m, :], zp[:])
                  if skip_ag:
                      zcur[br] = (z_sb, None)
                      return
                  if drz:
                      zb = dram.tile([SM2, P, 2 * F], FP8E4, name="zb")
                      nc.sync.dma_start(
                          zb.rearrange("m p (j f) -> p m j f", j=2), z_sb[:])
                      zf = dram.tile([NC * SM2, P, 2 * F], FP8E4, name="zf",
                                     addr_space="Shared")
                  else:
                      zb = dram.tile([SM, P, F], BF16, name="zb")
                      nc.sync.dma_start(zb.rearrange("m p f -> p m f"), z_sb[:])
                      zf = dram.tile([NC * SM, P, F], BF16, name="zf",
                                     addr_space="Shared")
                  nc.gpsimd.collective_compute(
                      "AllGather", mybir.AluOpType.bypass,
                      replica_groups=RG, ins=[zb.opt()], outs=[zf.opt()],
                  )
                  zcur[br] = (z_sb, zf)

              def agg_step(br, i):
                  """aggT_shard accumulated over all nodes; relu+bias; max."""
                  z_sb, zf = zcur[br]
                  h_t = const.tile([P, FC, S], BF16, name=f"ht{br}_{i}")
                  ht[br][i] = h_t
                  aps = [[psa.tile([P, 512], F32, name="aps", tag="psa")
                          for _ in NT] for _ in range(FC)]
                  drl = dr or (br, i) in dr_li
                  fp8 = (br, i) in fp8_li and not drl
                  if dr:
                      a_src = at_d[br]
                  elif drl:
                      a_src = at_d[br][2]
                  else:
                      a_src = at_d[br][1] if fp8 else at_d[br][0]
                  a_dt = FP8E4 if drl else (FP8E3 if fp8 else BF16)
                  if skip_at:
                      if drl:
                          atk0 = atp.tile([P, 2, S], a_dt, name="atk0",
                                          tag="atk0", bufs=1)
                      else:
                          atk0 = atp.tile([P, S], a_dt, name="atk0",
                                          tag="atk0", bufs=1)
                      nc.sync.dma_start(atk0[:], a_src[0])
                  # chunk KC-1 covers only padding rows (>=10112): all-zero
                  nk = KC2 if drl else KC - 1
                  for k in range(nk):
                      if skip_at:
                          atk = atk0
                      elif drl:
                          atk = atp.tile([P, 2, S], a_dt, name="atk")
                          nc.sync.dma_start(atk[:], a_src[k])
                      else:
                          atk = atp.tile([P, S], a_dt, name="atk")
                          nc.sync.dma_start(atk[:], a_src[k])
                      if skip_z or skip_ag:
                          zk = None
                      elif drl:
                          zk = zkp.tile([P, 2, F], FP8E4, name="zk")
                          nc.sync.dma_start(zk[:], zf[k].rearrange("p (j f) -> p j f", j=2))
                      else:
                          zk = zkp.tile([P, F], BF16, name="zk")
                          nc.sync.dma_start(zk[:], zf[k])
                      for fc in range(FC):
                          for n, (off, sz) in enumerate(NT):
                              if drl:
                                  lhsT = (z_sb[:, 0, :, fc * P:(fc + 1) * P]
                                          if zk is None else
                                          zk[:, :, fc * P:(fc + 1) * P])
                                  nc.tensor.matmul(
                                      aps[fc][n][:, :sz],
                                      lhsT,
                                      atk[:, :, off:off + sz],
                                      start=(k == 0),
                                      stop=(k == nk - 1),
                                      perf_mode=mybir.MatmulPerfMode.DoubleRow,
                                  )
                              else:
                                  lhsT = (z_sb[:, 0, fc * P:(fc + 1) * P]
                                          if zk is None else
                                          zk[:, fc * P:(fc + 1) * P])
                                  nc.tensor.matmul(
                                      aps[fc][n][:, :sz],
                                      lhsT,
                                      atk[:, off:off + sz],
                                      start=(k == 0),
                                      stop=(k == nk - 1),
                                  )
                  act_scale = 1.0 / (ASCALE * ZSCALE) if drl else 1.0
                  for fc in range(FC):
                      for n, (off, sz) in enumerate(NT):
                          nc.scalar.activation(
                              h_t[:, fc, off:off + sz], aps[fc][n][:, :sz],
                              AF.Relu, bias=bt_sb[br][:, i, fc:fc + 1],
                              scale=act_scale,
                          )
                  nc.vector.reduce_max(
                      mx_sb[:, br * L + i: br * L + i + 1], h_t[:],
                      axis=mybir.AxisListType.XY,
                  )

              # ---- pipelined layers: branch 1 leads ----
              z_step(1, 0)
              z_step(0, 0)
              for i in range(L):
                  agg_step(1, i)
                  if i + 1 < L:
                      z_step(1, i + 1)
                  agg_step(0, i)
                  if i + 1 < L:
                      z_step(0, i + 1)

              if not only_layers:
                # ---- per-branch attention (AllReduce max + tiny MLP) ----
                attb = [None, None]

                def attention(br):
                    mxb = dram.tile([P, L], F32, name="mxb")
                    nc.sync.dma_start(mxb[:], mx_sb[:, br * L:(br + 1) * L])
                    mxr = dram.tile([P, L], F32, name="mxr", addr_space="Shared")
                    nc.gpsimd.collective_compute(
                        "AllReduce", mybir.AluOpType.max,
                        replica_groups=RG, ins=[mxb.opt()], outs=[mxr.opt()],
                    )
                    return mxr

                def att_mlp(br, mxr):
                    mrow = sb.tile([1, L, P], F32, name="mrow")
                    nc.sync.dma_start(mrow[:], mxr.rearrange("p i -> () i p"))
                    att0 = sb.tile([1, L], F32, name="att0")
                    nc.vector.reduce_max(att0[:], mrow[:],
                                         axis=mybir.AxisListType.X)
                    a0d = dram.tile([1, L], F32, name="a0d")
                    nc.sync.dma_start(a0d[:], att0[:])
                    a0col = sb.tile([L, 1], F32, name="a0col")
                    nc.sync.dma_start(a0col[:], a0d.rearrange("() c -> c ()"))
                    p1 = psz.tile([5 * L, 1], F32, name="p1", tag="psz")
                    nc.tensor.matmul(p1[:], fc1wt_sb[br][:], a0col[:],
                                     start=True, stop=True)
                    y1 = sb.tile([5 * L, 1], F32, name="y1")
                    nc.scalar.activation(y1[:], p1[:], AF.Relu, bias=fc1b_sb[br][:])
                    p2 = psz.tile([L, 1], F32, name="p2", tag="psz")
                    nc.tensor.matmul(p2[:], fc2wt_sb[br][:], y1[:],
                                     start=True, stop=True)
                    attc = sb.tile([L, 1], F32, name="attc")
                    nc.scalar.activation(attc[:], p2[:], AF.Sigmoid,
                                         bias=fc2b_sb[br][:])
                    attd = dram.tile([L, 1], F32, name="attd")
                    nc.sync.dma_start(attd[:], attc[:])
                    attrow = sb.tile([1, L], F32, name="attrow")
                    nc.sync.dma_start(attrow[:], attd.rearrange("c () -> () c"))
                    pb = psz.tile([P, L], F32, name="pb", tag="psz")
                    nc.tensor.matmul(pb[:], ones_sb[:], attrow[:],
                                     start=True, stop=True)
                    ab = sb.tile([P, L], F32, name="attb")
                    nc.vector.tensor_copy(ab[:], pb[:])
                    attb[br] = ab

                def conv(br):
                    scw = const.tile([P, L, FC, OC], BF16, name=f"scw{br}")
                    for c in range(L):
                        for fc in range(FC):
                            nc.vector.tensor_scalar_mul(
                                scw[:, c, fc, :], cwt_sb[br][:, c, fc, :],
                                attb[br][:, c:c + 1],
                            )
                    o_t = const.tile([P, S], BF16, name=f"oxt{br}")
                    for n, (off, sz) in enumerate(NT):
                        cps = psa.tile([P, 512], F32, name="cps", tag="psa")
                        for c in range(L):
                            for fc in range(FC):
                                nc.tensor.matmul(
                                    cps[:, :sz], scw[:, c, fc, :],
                                    ht[br][c][:, fc, off:off + sz],
                                    start=(c == 0 and fc == 0),
                                    stop=(c == L - 1 and fc == FC - 1),
                                )
                        nc.vector.tensor_scalar_add(
                            o_t[:, off:off + sz], cps[:, :sz], cb_sb[br][:]
                        )
                    return o_t

                mxr1 = attention(1)
                mxr0 = attention(0)
                att_mlp(1, mxr1)
                oyt = conv(1)
                oyb = dram.tile([P, S], BF16, name="oyb")
                nc.sync.dma_start(oyb[:], oyt[:])
                oyf = dram.tile([NC * P, S], BF16, name="oyf", addr_space="Shared")
                nc.gpsimd.collective_compute(
                    "AllGather", mybir.AluOpType.bypass,
                    replica_groups=RG, ins=[oyb.opt()], outs=[oyf.opt()],
                )
                att_mlp(0, mxr0)
                oxt = conv(0)

                # ---- final: out_shard = out_x_shard.T @ out_y_full ----
                kts = []
                for r in range(NC):
                    kt = ktp.tile([P, S], BF16, name="kt")
                    nc.sync.dma_start(kt[:], oyf[r * P:(r + 1) * P, :])
                    kts.append(kt)
                for m in range(SM):
                    ob = fop.tile([P, NPAD], FP16, name="ob")
                    for r in range(NC):
                        for n, (off, sz) in enumerate(NT):
                            fps = psa.tile([P, 512], F32, name="fps", tag="psa")
                            nc.tensor.matmul(
                                fps[:, :sz], oxt[:, m * P:(m + 1) * P],
                                kts[r][:, off:off + sz], start=True, stop=True,
                            )
                            if (r * len(NT) + n) % 2 == 0:
                                nc.vector.tensor_copy(
                                    ob[:, r * S + off: r * S + off + sz],
                                    fps[:, :sz])
                            else:
                                nc.scalar.activation(
                                    ob[:, r * S + off: r * S + off + sz],
                                    fps[:, :sz], AF.Copy)
                    nc.sync.dma_start(out_tgt[m * P:(m + 1) * P, :], ob[:])

            for _ in range(repeat):
                emit()
            if timing:
                done_sb = sb.tile([P, 2 * L], F32, name="done_sb")
                nc.vector.tensor_copy(done_sb[:], mx_sb[:])
                nc.sync.dma_start(done_d[:], done_sb[:])
    nc.compile()
    return nc


def _build_at(edges, ew):
    """Dense transposed normalized adjacency A_hat.T (x ASCALE), padded."""
    src = np.asarray(edges[0], dtype=np.int64)
    dst = np.asarray(edges[1], dtype=np.int64)
    w = np.asarray(ew, dtype=np.float64)
    deg = np.ones(N_NODE, dtype=np.float64)  # self loops, weight 1
    np.add.at(deg, dst, w)
    dinv = 1.0 / np.sqrt(deg)
    norm = (dinv[src] * w * dinv[dst]).astype(np.float32)
    at = np.zeros((NPAD, NPAD), dtype=np.float32)
    np.add.at(at, (src, dst), norm)
    ii = np.arange(N_NODE)
    at[ii, ii] += (dinv * dinv).astype(np.float32)
    return at * ASCALE


def _prep_branch(x, ew, W, b, cw, cb, f1w, f1b, f2w, f2b, edges, br=0, dr=None):
    dr_li = frozenset() if (DR if dr is None else dr) else frozenset(DR_LI)
    at = np.clip(_build_at(edges, ew), 0.0, 28.0)
    xp = np.zeros((NPAD, F), dtype=np.float32)
    xp[:N_NODE] = np.asarray(x, dtype=np.float32)
    x0t = np.ascontiguousarray(xp.T).astype(BF)          # [F, NPAD]
    wf = np.asarray(W, np.float32).copy()
    for i in range(L):
        # DR layers: z stays in e4m3's normal range (boosted), and the
        # combined A*z scale is divided out in the relu activation.
        wf[i] *= ZSCALE if (br, i) in dr_li else (1.0 / ASCALE)
    wq = wf.reshape(L, FC, P, F).astype(BF)
    bt = np.asarray(b, np.float32).reshape(L, FC, P).astype(np.float32)
    cwt = np.ascontiguousarray(
        np.asarray(cw, np.float32)[:, :, :, 0].transpose(1, 2, 0)
    ).reshape(L, FC, P, OC).astype(BF)                   # [c, f, oc]
    cbq = np.asarray(cb, np.float32).reshape(P, 1)
    f1wt = np.ascontiguousarray(np.asarray(f1w, np.float32).T)  # [5,25]
    f1bq = np.asarray(f1b, np.float32).reshape(5 * L, 1)
    f2wt = np.ascontiguousarray(np.asarray(f2w, np.float32).T)  # [25,5]
    f2bq = np.asarray(f2b, np.float32).reshape(L, 1)
    return at, x0t, wq, bt, cwt, cbq, f1wt, f1bq, f2wt, f2bq


def _shard_at(at, k, dtype, dr=None):
    """Per-core A_hat.T slice in the on-chip chunk layout."""
    dr = DR if dr is None else dr
    sl = slice(k * S, (k + 1) * S)
    a = np.ascontiguousarray(at[:, sl]).astype(dtype)
    if dr:
        # pair (p, j) <-> global row kk*256 + j*128 + p
        return np.ascontiguousarray(
            a.reshape(KC2, 2, P, S).transpose(0, 2, 1, 3))
    return a.reshape(KC, P, S)


def _make_in_maps(inputs, dr=None):
    dr = DR if dr is None else dr
    br0 = _prep_branch(
        inputs["x_m"], inputs["w_m"], inputs["Wx"], inputs["bx"],
        inputs["cnnx_w"], inputs["cnnx_b"], inputs["fc1x_w"], inputs["fc1x_b"],
        inputs["fc2x_w"], inputs["fc2x_b"], inputs["edges_m"], br=0, dr=dr,
    )
    br1 = _prep_branch(
        inputs["x_d"], inputs["w_d"], inputs["Wy"], inputs["by"],
        inputs["cnny_w"], inputs["cnny_b"], inputs["fc1y_w"], inputs["fc1y_b"],
        inputs["fc2y_w"], inputs["fc2y_b"], inputs["edges_d"], br=1, dr=dr,
    )

    in_maps = []
    for k in range(NC):
        m = {}
        for br, (at, x0t, wq, bt, cwt, cbq, f1wt, f1bq, f2wt, f2bq) in enumerate(
            (br0, br1)
        ):
            sl = slice(k * S, (k + 1) * S)
            fp8_li = frozenset(FP8_LI)
            dr_li = frozenset() if dr else frozenset(DR_LI)
            all_li = {(b_, i_) for b_ in range(2) for i_ in range(L)}
            if dr:
                m[f"at{br}"] = _shard_at(at, k, E4, dr=True)
            else:
                if len(all_li - fp8_li - dr_li) > 0:
                    m[f"at{br}"] = _shard_at(at, k, BF, dr=False)
                if len(fp8_li) > 0:
                    m[f"at8_{br}"] = _shard_at(at, k, E3, dr=False)
                if len(dr_li) > 0:
                    m[f"atdr{br}"] = _shard_at(at, k, E4, dr=True)
            m[f"x0t{br}"] = np.ascontiguousarray(x0t[:, sl]).reshape(FC, P, S)
            m[f"w{br}"] = wq
            m[f"bt{br}"] = bt
            m[f"cwt{br}"] = cwt
            m[f"cb{br}"] = cbq
            m[f"fc1wt{br}"] = f1wt
            m[f"fc1b{br}"] = f1bq
            m[f"fc2wt{br}"] = f2wt
            m[f"fc2b{br}"] = f2bq
        in_maps.append(m)
    return in_maps


def kernel(**inputs):
    if "nc" not in _CACHE:
        _CACHE["nc"] = _build()
    nc = _CACHE["nc"]
    in_maps = _make_in_maps(inputs)
    res = run_bass_kernel_spmd(nc, in_maps, core_ids=list(range(NC)))
    full = np.concatenate([res.results[k]["out"] for k in range(NC)], axis=0)
    return np.ascontiguousarray(full[:N_NODE, :N_NODE]).astype(np.float32)
